# revision 30
# baseline (speedup 1.0000x reference)
"""GQA (grouped-query attention) Trainium2 kernel, 8-core SPMD.

Sharding: core = (batch b, query-quarter q4). Host rotates x^T columns (and
the mask bias) per core so the core's OWN query quarter is always chunk n=0
-- the program is core-independent (SPMD) and chunk 0 doubles as xTq.

Per core:
  - loads x^T once (4 chunks of 512 seq cols); each chunk feeds the K
    projection (moving operand), V projection (stationary slices) and, for
    chunk 0, the Q projection (moving operand),
  - projects k^T and v (+ones column) for the full sequence,
  - attention loop over 16 head-pairs x 16 key tiles: scores^T = k @ q^T,
    exp on ACT (fused scale+mask-bias), AV matmul whose ones column yields
    the softmax denominator; Q projection for head-pair hp+2 is interleaved
    into the back half of each hp pass to fill PE slack under the Act-bound
    exp stream,
  - normalizes via DMA partition-broadcast of the reciprocal row,
  - applies out-proj transposed: out^T = o_w^T @ ctx^T.
Host assembles the quarters back to [B, S, D].

Everything is laid out "transposed" (feature dim on partitions, sequence on
the free axis) so softmax reductions, biases and the key mask are all
free-axis / per-partition ops and no on-chip transposes are needed.
Matmul data is bf16 (full PE rate); accumulation is fp32 in PSUM.

Head pairing: PE matmul needs lhsT/rhs at the same base partition.  k^T for
group g lives at partition base (g%2)*64, so q^T tiles pair one even-group
head (partitions 0:64) with one odd-group head (64:128) via a host-side
permutation of q_w columns / o_w rows.
"""

import numpy as np
import ml_dtypes

import concourse.bass as bass
import concourse.mybir as mybir
import concourse.tile as tile
from concourse import bacc
from concourse import bass_utils

B, S, D = 2, 2048, 2048
H, G = 32, 8
HD = D // H            # 64
HPG = H // G           # 4
KV = G * HD            # 512
P = 128
SQ = S // 4            # 512 queries per core
NK = D // P            # 16 contraction chunks
NSK = S // P           # 16 key tiles
N_CORES = 8

HEADS_E = [h for h in range(H) if (h // HPG) % 2 == 0]
HEADS_O = [h for h in range(H) if (h // HPG) % 2 == 1]

BF16 = mybir.dt.bfloat16
F32 = mybir.dt.float32
EXP = mybir.ActivationFunctionType.Exp
ADD = mybir.AluOpType.add
MULT = mybir.AluOpType.mult

_CACHE = {}


def _build():
    nc = bacc.Bacc("TRN2", target_bir_lowering=False, debug=False,
                   num_devices=N_CORES)

    xT_d = nc.dram_tensor("xT", [D, S], BF16, kind="ExternalInput")
    qw_d = nc.dram_tensor("qw", [D, D], BF16, kind="ExternalInput")
    kw_d = nc.dram_tensor("kw", [D, KV], BF16, kind="ExternalInput")
    vw_d = nc.dram_tensor("vw", [D, KV], BF16, kind="ExternalInput")
    ow_d = nc.dram_tensor("ow", [D, D], BF16, kind="ExternalInput")
    qb_d = nc.dram_tensor("qb", [D, 1], F32, kind="ExternalInput")
    kb_d = nc.dram_tensor("kb", [KV, 1], F32, kind="ExternalInput")
    ob_d = nc.dram_tensor("ob", [D, 1], F32, kind="ExternalInput")
    mb_d = nc.dram_tensor("mb", [S, 1], F32, kind="ExternalInput")
    vbo_d = nc.dram_tensor("vbo", [P, G * (HD + 1)], BF16, kind="ExternalInput")
    outT_d = nc.dram_tensor("outT", [D, SQ], F32, kind="ExternalOutput")

    with tile.TileContext(nc) as tc:
        with (
            tc.tile_pool(name="resid", bufs=1) as resid,
            tc.tile_pool(name="xs", bufs=2) as xs_pool,
            tc.tile_pool(name="qwp", bufs=3) as qw_pool,
            tc.tile_pool(name="qtp", bufs=3) as qt_pool,
            tc.tile_pool(name="owp", bufs=2) as ow_pool,
            tc.tile_pool(name="attn", bufs=4) as attn_pool,
            tc.tile_pool(name="ev", bufs=2) as ev_pool,
            tc.tile_pool(name="bc", bufs=4) as bc_pool,
            tc.tile_pool(name="cr", bufs=4) as cr_pool,
            tc.tile_pool(name="rc", bufs=4) as rc_pool,
            tc.tile_pool(name="psA", bufs=4, space="PSUM") as psA,
            tc.tile_pool(name="psS", bufs=2, space="PSUM") as psS,
        ):
            # ---- resident tiles ----
            xq = resid.tile([P, NK, SQ], BF16)       # own-quarter x^T (chunk 0)
            kw_sb = resid.tile([P, NK, KV], BF16)
            vw_sb = resid.tile([P, NK, KV], BF16)
            kT_all = resid.tile([P, KV // P, S], BF16)      # k^T  [kv-col, Sk]
            v_ones = resid.tile([P, NSK, G, HD + 1], BF16)  # v (+ones) per Sk tile
            ctx_all = resid.tile([P, NK, SQ], BF16)         # ctx^T stacked head pairs

            # startup loads, k-chunk interleaved so kproj(0) starts early;
            # the first chunks are split finest so the first matmul can begin
            # after ~2 small DMAs instead of one large one.
            splits = [(0, 1), (1, 2), (2, 4), (4, 8), (8, 12), (12, 16)]
            for k0, k1 in splits:
                nc.sync.dma_start(
                    xq[:, k0:k1, :],
                    xT_d.ap()[k0 * P:k1 * P, 0:SQ]
                    .rearrange("(k p) c -> p k c", p=P))
                nc.sync.dma_start(
                    kw_sb[:, k0:k1, :],
                    kw_d.ap()[k0 * P:k1 * P, :]
                    .rearrange("(k p) c -> p k c", p=P))
            kb_sb = resid.tile([P, KV // P], F32)
            nc.sync.dma_start(kb_sb[:], kb_d.ap().rearrange("(k p) one -> p (k one)", p=P))
            for c in range(2):
                nc.sync.dma_start(
                    vw_sb[:, 8 * c:8 * (c + 1), :],
                    vw_d.ap()[8 * c * P:8 * (c + 1) * P, :]
                    .rearrange("(k p) c -> p k c", p=P))
            mb_sb = resid.tile([P, NSK], F32)
            nc.sync.dma_start(mb_sb[:], mb_d.ap().rearrange("(i p) one -> p (i one)", p=P))
            vbo_sb = resid.tile([P, G, HD + 1], BF16)
            nc.sync.dma_start(vbo_sb[:], vbo_d.ap().rearrange("p (g c) -> p g c", g=G))
            qb_sb = resid.tile([P, NK], F32)
            nc.sync.dma_start(qb_sb[:], qb_d.ap().rearrange("(k p) one -> p (k one)", p=P))
            ob_sb = resid.tile([P, NK], F32)
            nc.sync.dma_start(ob_sb[:], ob_d.ap().rearrange("(k p) one -> p (k one)", p=P))

            # ---- K/V projection phase: one x^T chunk serves both ----
            def emit_kproj(n, xs):
                ps_k = [psA.tile([P, SQ], F32, tag="ps", name=f"ps_k{n}_{m}")
                        for m in range(4)]
                for k in range(NK):
                    for m in range(4):
                        nc.tensor.matmul(ps_k[m][:], kw_sb[:, k, m * P:(m + 1) * P],
                                         xs[:, k, :], start=(k == 0), stop=(k == NK - 1))
                for m in range(4):
                    nc.vector.tensor_scalar_add(kT_all[:, m, n * SQ:(n + 1) * SQ],
                                                ps_k[m][:], kb_sb[:, m:m + 1])

            def emit_vproj(i, xs):
                j = i % 4  # 128-col slice within the 512-col chunk
                ps_v = psS.tile([P, KV], F32, tag="sc", name=f"ps_v{i}")
                for k in range(NK):
                    nc.tensor.matmul(ps_v[:], xs[:, k, j * P:(j + 1) * P],
                                     vw_sb[:, k, :], start=(k == 0), stop=(k == NK - 1))
                nc.vector.tensor_tensor(
                    v_ones[:, i, :, 0:HD],
                    ps_v[:].rearrange("p (g c) -> p g c", g=G),
                    vbo_sb[:, :, 0:HD], op=ADD)
                nc.vector.tensor_copy(v_ones[:, i, :, HD:HD + 1], vbo_sb[:, :, HD:HD + 1])

            for n in range(4):
                if n == 0:
                    xs = xq
                else:
                    xs = xs_pool.tile([P, NK, SQ], BF16, tag="xs", name=f"xs{n}")
                    nc.sync.dma_start(xs[:], xT_d.ap()[:, n * SQ:(n + 1) * SQ]
                                      .rearrange("(k p) c -> p k c", p=P))
                emit_kproj(n, xs)
                for i in range(4 * n, 4 * n + 4):
                    emit_vproj(i, xs)

            # ---- Q projection (head-pair granularity) ----
            qT = {}  # hp -> sbuf tile [P, SQ]

            def emit_qproj_mm(hp):
                """DMA q_w tile + 16 accumulating matmuls into a psA bank."""
                qwt = qw_pool.tile([P, NK, P], BF16, tag="qw", name=f"qwt{hp}")
                nc.sync.dma_start(qwt[:], qw_d.ap()[:, hp * P:(hp + 1) * P]
                                  .rearrange("(k p) c -> p k c", p=P))
                ps_q = psA.tile([P, SQ], F32, tag="ps", name=f"ps_q{hp}")
                mms = []
                for k in range(NK):
                    mms.append((ps_q, qwt, k))
                return ps_q, qwt, mms

            def emit_qp_step(ps_q, qwt, k):
                nc.tensor.matmul(ps_q[:], qwt[:, k, :], xq[:, k, :],
                                 start=(k == 0), stop=(k == NK - 1))

            def emit_qproj_fin(hp, ps_q):
                qt = qt_pool.tile([P, SQ], BF16, tag="qt", name=f"qt{hp}")
                nc.vector.tensor_scalar_add(qt[:], ps_q[:], qb_sb[:, hp:hp + 1])
                qT[hp] = qt

            # hp 0 and 1 fully before attention
            for hp in range(2):
                ps_q, qwt, _ = emit_qproj_mm(hp)
                for k in range(NK):
                    emit_qp_step(ps_q, qwt, k)
                emit_qproj_fin(hp, ps_q)

            # ---- attention ----
            def emit_scores(hp, i):
                gA = HEADS_E[hp] // HPG
                gB = HEADS_O[hp] // HPG
                sc = psS.tile([P, 2 * SQ], F32, tag="sc", name=f"sc{hp}_{i}")
                nc.tensor.matmul(sc[:, 0:SQ],
                                 kT_all[0:HD, gA // 2, i * P:(i + 1) * P],
                                 qT[hp][0:HD, :], start=True, stop=True)
                nc.tensor.matmul(sc[:, SQ:2 * SQ],
                                 kT_all[HD:2 * HD, gB // 2, i * P:(i + 1) * P],
                                 qT[hp][HD:2 * HD, :], start=True, stop=True)
                return sc

            def emit_normalize(hp, ctx0, ctx1):
                # normalize: ctx[c, q] / Z[q]  (Z = ones-column row).
                # Copy psum out FIRST (cheap) so the banks free early for the
                # next head-pair; the broadcast+mult then run off SBUF.
                for half, cps in ((0, ctx0), (1, ctx1)):
                    craw = cr_pool.tile([HD, SQ], F32, tag="cr")
                    nc.vector.tensor_copy(craw[:], cps[0:HD, :])
                    zrow = rc_pool.tile([1, SQ], F32, tag="rc")
                    nc.vector.reciprocal(zrow[:], cps[HD:HD + 1, :])
                    bcv = bc_pool.tile([HD, SQ], F32, tag="bc")
                    nc.gpsimd.partition_broadcast(bcv[:], zrow[:])
                    nc.vector.tensor_tensor(
                        ctx_all[half * HD:(half + 1) * HD, hp, :],
                        craw[:], bcv[:], op=MULT)

            # Flat unit stream over (hp, i).  Per unit: exp(u) on ACT, then
            # scores(u+1) on PE (its psum slot was just freed by exp(u-1)),
            # THEN the AV pair of the PREVIOUS unit, then one interleaved
            # q-proj matmul.  Emitting AV one unit late keeps the freshly
            # unblocked scores matmul at the PE queue head, breaking the
            # sem->AV->scores->sem cycle that would otherwise pace the ACT
            # stream slower than exp itself.
            sc_next = emit_scores(0, 0)
            ctx = {}     # hp -> (ctx0, ctx1)
            qp_cur = None
            pend = None  # (at, i, gA, gB, hp) awaiting AV emission

            def emit_av(at, i, gA, gB, hp):
                ctx0, ctx1 = ctx[hp]
                nc.tensor.matmul(ctx0[0:HD + 1, :],
                                 v_ones[:, i, gA, :], at[:, 0:SQ],
                                 start=(i == 0), stop=(i == NSK - 1))
                nc.tensor.matmul(ctx1[0:HD + 1, :],
                                 v_ones[:, i, gB, :], at[:, SQ:2 * SQ],
                                 start=(i == 0), stop=(i == NSK - 1))

            for u in range(NK * NSK):
                hp, i = divmod(u, NSK)
                gA = HEADS_E[hp] // HPG
                gB = HEADS_O[hp] // HPG
                if i == 0:
                    ctx[hp] = (psA.tile([P, SQ], F32, tag="ps", name=f"ctx0_{hp}"),
                               psA.tile([P, SQ], F32, tag="ps", name=f"ctx1_{hp}"))
                    if hp + 2 < NK:
                        qp_cur = emit_qproj_mm(hp + 2)
                    else:
                        qp_cur = None
                sc = sc_next
                at = attn_pool.tile([P, 2 * SQ], BF16, tag="at")
                nc.scalar.activation(at[:], sc[:], EXP,
                                     bias=mb_sb[:, i:i + 1], scale=0.125)
                if u + 1 < NK * NSK:
                    nhp, ni = divmod(u + 1, NSK)
                    sc_next = emit_scores(nhp, ni)
                if pend is not None:
                    emit_av(*pend)
                    if pend[1] == NSK - 1:   # that AV closed head-pair hp-1
                        php = pend[4]
                        emit_normalize(php, *ctx.pop(php))
                        qT.pop(php, None)
                pend = (at, i, gA, gB, hp)
                if qp_cur is not None:
                    emit_qp_step(qp_cur[0], qp_cur[1], i)
                    if i == NSK - 1:
                        emit_qproj_fin(hp + 2, qp_cur[0])
            emit_av(*pend)
            emit_normalize(NK - 1, *ctx.pop(NK - 1))

            # ---- out projection (transposed): out^T = o_w^T @ ctx^T ----
            for mh in range(8):  # half-groups of 256 output cols
                owt = ow_pool.tile([P, NK, 2 * P], BF16, tag="ow", name=f"owt{mh}")
                nc.sync.dma_start(owt[:], ow_d.ap()[:, mh * 2 * P:(mh + 1) * 2 * P]
                                  .rearrange("(k p) c -> p k c", p=P))
                ps_o = [psA.tile([P, SQ], F32, tag="ps", name=f"ps_o{mh}_{mj}")
                        for mj in range(2)]
                for k in range(NK):
                    for mj in range(2):
                        nc.tensor.matmul(ps_o[mj][:], owt[:, k, mj * P:(mj + 1) * P],
                                         ctx_all[:, k, :],
                                         start=(k == 0), stop=(k == NK - 1))
                for mj in range(2):
                    m = mh * 2 + mj
                    ot = ev_pool.tile([P, SQ], F32, tag="ot")
                    nc.vector.tensor_scalar_add(ot[:], ps_o[mj][:], ob_sb[:, m:m + 1])
                    nc.sync.dma_start(outT_d.ap()[m * P:(m + 1) * P, :], ot[:])

    nc.compile()
    return nc


def _get_nc():
    if "nc" not in _CACHE:
        _CACHE["nc"] = _build()
    return _CACHE["nc"]


def prep_in_maps(x, mask, q_w, q_b, k_w, k_b, v_w, v_b, o_w, o_b):
    """Host-side sharding: returns the 8 per-core input dicts."""
    bf = ml_dtypes.bfloat16
    x = np.asarray(x, np.float32)
    mask = np.asarray(mask)
    q_w = np.asarray(q_w, np.float32)
    q_b = np.asarray(q_b, np.float32)
    o_w = np.asarray(o_w, np.float32)
    v_b = np.asarray(v_b, np.float32)

    # head permutation: tile hp = (HEADS_E[hp], HEADS_O[hp])
    col_perm = np.zeros(D, np.int64)
    for hp in range(NK):
        col_perm[hp * P:hp * P + HD] = np.arange(HEADS_E[hp] * HD,
                                                 (HEADS_E[hp] + 1) * HD)
        col_perm[hp * P + HD:(hp + 1) * P] = np.arange(HEADS_O[hp] * HD,
                                                       (HEADS_O[hp] + 1) * HD)
    qw = np.ascontiguousarray(q_w[:, col_perm]).astype(bf)
    qb = np.ascontiguousarray(q_b[col_perm]).reshape(D, 1)
    ow = np.ascontiguousarray(o_w[col_perm, :]).astype(bf)

    kw = np.asarray(k_w, np.float32).astype(bf)
    vw = np.asarray(v_w, np.float32).astype(bf)
    kb = np.asarray(k_b, np.float32).reshape(KV, 1)
    ob = np.asarray(o_b, np.float32).reshape(D, 1)

    vbo = np.zeros((P, G * (HD + 1)), np.float32)
    for g in range(G):
        vbo[:, g * (HD + 1):g * (HD + 1) + HD] = v_b[g * HD:(g + 1) * HD][None, :]
        vbo[:, g * (HD + 1) + HD] = 1.0
    vbo = vbo.astype(bf)

    xT = [np.ascontiguousarray(x[b].T).astype(bf) for b in range(B)]
    mbias = [np.where(np.asarray(mask[b]) == 0, np.float32(-30000.0),
                      np.float32(0.0)) for b in range(B)]

    in_maps = []
    for c in range(N_CORES):
        b, q4 = c // 4, c % 4
        # rotate so own query quarter is key-chunk 0
        xTr = np.ascontiguousarray(np.roll(xT[b], -q4 * SQ, axis=1))
        mbr = np.ascontiguousarray(np.roll(mbias[b], -q4 * SQ)).reshape(S, 1)
        in_maps.append({
            "xT": xTr,
            "qw": qw, "kw": kw, "vw": vw, "ow": ow,
            "qb": qb, "kb": kb, "ob": ob,
            "mb": mbr, "vbo": vbo,
        })
    return in_maps


def kernel(x, mask, q_w, q_b, k_w, k_b, v_w, v_b, o_w, o_b):
    in_maps = prep_in_maps(x, mask, q_w, q_b, k_w, k_b, v_w, v_b, o_w, o_b)
    nc = _get_nc()
    res = bass_utils.run_bass_kernel_spmd(nc, in_maps, core_ids=list(range(N_CORES)))
    out = np.empty((B, S, D), np.float32)
    for c in range(N_CORES):
        b, q4 = c // 4, c % 4
        out[b, q4 * SQ:(q4 + 1) * SQ, :] = res.results[c]["outT"].T
    return out


# revision 36
# speedup vs baseline: 8.5499x; 8.5499x over previous
"""GQA (grouped-query attention) Trainium2 kernel, 8-core SPMD.

Sharding: core = (batch b, query-quarter q4). Host rotates x^T columns (and
the mask bias) per core so the core's OWN query quarter is always chunk n=0
-- the program is core-independent (SPMD) and chunk 0 doubles as xTq.

Per core:
  - loads x^T once (4 chunks of 512 seq cols); each chunk feeds the K
    projection (moving operand), V projection (stationary slices) and, for
    chunk 0, the Q projection (moving operand),
  - projects k^T and v (+ones column) for the full sequence,
  - attention loop over 16 head-pairs x 16 key tiles: scores^T = k @ q^T,
    exp on ACT (fused scale+mask-bias), AV matmul whose ones column yields
    the softmax denominator; Q projection for head-pair hp+2 is interleaved
    into the back half of each hp pass to fill PE slack under the Act-bound
    exp stream,
  - normalizes via DMA partition-broadcast of the reciprocal row,
  - applies out-proj transposed: out^T = o_w^T @ ctx^T.
Host assembles the quarters back to [B, S, D].

Everything is laid out "transposed" (feature dim on partitions, sequence on
the free axis) so softmax reductions, biases and the key mask are all
free-axis / per-partition ops and no on-chip transposes are needed.
Matmul data is bf16 (full PE rate); accumulation is fp32 in PSUM.

Head pairing: PE matmul needs lhsT/rhs at the same base partition.  k^T for
group g lives at partition base (g%2)*64, so q^T tiles pair one even-group
head (partitions 0:64) with one odd-group head (64:128) via a host-side
permutation of q_w columns / o_w rows.
"""

import numpy as np
import ml_dtypes

import concourse.bass as bass
import concourse.mybir as mybir
import concourse.tile as tile
from concourse import bacc
from concourse import bass_utils

B, S, D = 2, 2048, 2048
H, G = 32, 8
HD = D // H            # 64
HPG = H // G           # 4
KV = G * HD            # 512
P = 128
SQ = S // 4            # 512 queries per core
NK = D // P            # 16 contraction chunks
NSK = S // P           # 16 key tiles
N_CORES = 8

HEADS_E = [h for h in range(H) if (h // HPG) % 2 == 0]
HEADS_O = [h for h in range(H) if (h // HPG) % 2 == 1]

BF16 = mybir.dt.bfloat16
F32 = mybir.dt.float32
EXP = mybir.ActivationFunctionType.Exp
ADD = mybir.AluOpType.add
MULT = mybir.AluOpType.mult

_CACHE = {}


def _build():
    nc = bacc.Bacc("TRN2", target_bir_lowering=False, debug=False,
                   num_devices=N_CORES)

    xT_d = nc.dram_tensor("xT", [D, S], BF16, kind="ExternalInput")
    qw_d = nc.dram_tensor("qw", [D, D], BF16, kind="ExternalInput")
    kw_d = nc.dram_tensor("kw", [D, KV], BF16, kind="ExternalInput")
    vw_d = nc.dram_tensor("vw", [D, KV], BF16, kind="ExternalInput")
    ow_d = nc.dram_tensor("ow", [D, D], BF16, kind="ExternalInput")
    qb_d = nc.dram_tensor("qb", [D, 1], F32, kind="ExternalInput")
    kb_d = nc.dram_tensor("kb", [KV, 1], F32, kind="ExternalInput")
    ob_d = nc.dram_tensor("ob", [D, 1], F32, kind="ExternalInput")
    mb_d = nc.dram_tensor("mb", [S, 1], F32, kind="ExternalInput")
    vbo_d = nc.dram_tensor("vbo", [P, G * (HD + 1)], BF16, kind="ExternalInput")
    boot_d = nc.dram_tensor("boot", [P, 2 * SQ], BF16, kind="ExternalInput")
    outT_d = nc.dram_tensor("outT", [D, SQ], F32, kind="ExternalOutput")

    with tile.TileContext(nc) as tc:
        with (
            tc.tile_pool(name="resid", bufs=1) as resid,
            tc.tile_pool(name="xs", bufs=2) as xs_pool,
            tc.tile_pool(name="qwp", bufs=3) as qw_pool,
            tc.tile_pool(name="qtp", bufs=3) as qt_pool,
            tc.tile_pool(name="owp", bufs=2) as ow_pool,
            tc.tile_pool(name="attn", bufs=4) as attn_pool,
            tc.tile_pool(name="ev", bufs=2) as ev_pool,
            tc.tile_pool(name="bc", bufs=4) as bc_pool,
            tc.tile_pool(name="cr", bufs=4) as cr_pool,
            tc.tile_pool(name="rc", bufs=4) as rc_pool,
            tc.tile_pool(name="psA", bufs=4, space="PSUM") as psA,
            tc.tile_pool(name="psS", bufs=2, space="PSUM") as psS,
        ):
            # ---- resident tiles ----
            xq = resid.tile([P, NK, SQ], BF16)       # own-quarter x^T (chunk 0)
            kw_sb = resid.tile([P, NK, KV], BF16)
            vw_sb = resid.tile([P, NK, KV], BF16)
            kT_all = resid.tile([P, KV // P, S], BF16)      # k^T  [kv-col, Sk]
            v_ones = resid.tile([P, NSK, G, HD + 1], BF16)  # v (+ones) per Sk tile
            ctx_all = resid.tile([P, NK, SQ], BF16)         # ctx^T stacked head pairs

            # boot tile: x^T k-chunk 0 + k_w k-chunk 0 in ONE DMA, so the
            # first matmul waits on a single HWDGE+sem chain, not two.
            boot_sb = resid.tile([P, 2, SQ], BF16)
            nc.sync.dma_start(boot_sb[:],
                              boot_d.ap().rearrange("p (t c) -> p t c", t=2))

            # startup loads, k-chunk interleaved so kproj(0) starts early;
            # the first chunks are split finest so the first matmul can begin
            # after ~2 small DMAs instead of one large one.
            splits = [(0, 1), (1, 2), (2, 4), (4, 8), (8, 12), (12, 16)]
            for k0, k1 in splits:
                nc.sync.dma_start(
                    xq[:, k0:k1, :],
                    xT_d.ap()[k0 * P:k1 * P, 0:SQ]
                    .rearrange("(k p) c -> p k c", p=P))
                nc.sync.dma_start(
                    kw_sb[:, k0:k1, :],
                    kw_d.ap()[k0 * P:k1 * P, :]
                    .rearrange("(k p) c -> p k c", p=P))
            kb_sb = resid.tile([P, KV // P], F32)
            nc.sync.dma_start(kb_sb[:], kb_d.ap().rearrange("(k p) one -> p (k one)", p=P))
            for c in range(2):
                nc.sync.dma_start(
                    vw_sb[:, 8 * c:8 * (c + 1), :],
                    vw_d.ap()[8 * c * P:8 * (c + 1) * P, :]
                    .rearrange("(k p) c -> p k c", p=P))
            mb_sb = resid.tile([P, NSK], F32)
            nc.sync.dma_start(mb_sb[:], mb_d.ap().rearrange("(i p) one -> p (i one)", p=P))
            vbo_sb = resid.tile([P, G, HD + 1], BF16)
            nc.sync.dma_start(vbo_sb[:], vbo_d.ap().rearrange("p (g c) -> p g c", g=G))
            qb_sb = resid.tile([P, NK], F32)
            nc.sync.dma_start(qb_sb[:], qb_d.ap().rearrange("(k p) one -> p (k one)", p=P))
            ob_sb = resid.tile([P, NK], F32)
            nc.sync.dma_start(ob_sb[:], ob_d.ap().rearrange("(k p) one -> p (k one)", p=P))

            # ---- K/V projection phase: one x^T chunk serves both ----
            def emit_kproj(n, xs):
                ps_k = [psA.tile([P, SQ], F32, tag="ps", name=f"ps_k{n}_{m}")
                        for m in range(4)]
                for k in range(NK):
                    boot0 = (n == 0 and k == 0)
                    for m in range(4):
                        lhs = (boot_sb[:, 1, m * P:(m + 1) * P] if boot0
                               else kw_sb[:, k, m * P:(m + 1) * P])
                        rhs = boot_sb[:, 0, :] if boot0 else xs[:, k, :]
                        nc.tensor.matmul(ps_k[m][:], lhs, rhs,
                                         start=(k == 0), stop=(k == NK - 1))
                for m in range(4):
                    nc.vector.tensor_scalar_add(kT_all[:, m, n * SQ:(n + 1) * SQ],
                                                ps_k[m][:], kb_sb[:, m:m + 1])

            def emit_vproj(i, xs):
                j = i % 4  # 128-col slice within the 512-col chunk
                ps_v = psS.tile([P, KV], F32, tag="sc", name=f"ps_v{i}")
                for k in range(NK):
                    nc.tensor.matmul(ps_v[:], xs[:, k, j * P:(j + 1) * P],
                                     vw_sb[:, k, :], start=(k == 0), stop=(k == NK - 1))
                nc.vector.tensor_tensor(
                    v_ones[:, i, :, 0:HD],
                    ps_v[:].rearrange("p (g c) -> p g c", g=G),
                    vbo_sb[:, :, 0:HD], op=ADD)
                nc.vector.tensor_copy(v_ones[:, i, :, HD:HD + 1], vbo_sb[:, :, HD:HD + 1])

            for n in range(4):
                if n == 0:
                    xs = xq
                else:
                    xs = xs_pool.tile([P, NK, SQ], BF16, tag="xs", name=f"xs{n}")
                    nc.sync.dma_start(xs[:], xT_d.ap()[:, n * SQ:(n + 1) * SQ]
                                      .rearrange("(k p) c -> p k c", p=P))
                emit_kproj(n, xs)
                for i in range(4 * n, 4 * n + 4):
                    emit_vproj(i, xs)

            # ---- Q projection (head-pair granularity) ----
            qT = {}  # hp -> sbuf tile [P, SQ]

            def emit_qproj_mm(hp):
                """DMA q_w tile + 16 accumulating matmuls into a psA bank."""
                qwt = qw_pool.tile([P, NK, P], BF16, tag="qw", name=f"qwt{hp}")
                nc.sync.dma_start(qwt[:], qw_d.ap()[:, hp * P:(hp + 1) * P]
                                  .rearrange("(k p) c -> p k c", p=P))
                ps_q = psA.tile([P, SQ], F32, tag="ps", name=f"ps_q{hp}")
                mms = []
                for k in range(NK):
                    mms.append((ps_q, qwt, k))
                return ps_q, qwt, mms

            def emit_qp_step(ps_q, qwt, k):
                nc.tensor.matmul(ps_q[:], qwt[:, k, :], xq[:, k, :],
                                 start=(k == 0), stop=(k == NK - 1))

            def emit_qproj_fin(hp, ps_q):
                qt = qt_pool.tile([P, SQ], BF16, tag="qt", name=f"qt{hp}")
                nc.vector.tensor_scalar_add(qt[:], ps_q[:], qb_sb[:, hp:hp + 1])
                qT[hp] = qt

            # hp 0 and 1 fully before attention
            for hp in range(2):
                ps_q, qwt, _ = emit_qproj_mm(hp)
                for k in range(NK):
                    emit_qp_step(ps_q, qwt, k)
                emit_qproj_fin(hp, ps_q)

            # ---- attention ----
            def emit_scores(hp, i):
                gA = HEADS_E[hp] // HPG
                gB = HEADS_O[hp] // HPG
                sc = psS.tile([P, 2 * SQ], F32, tag="sc", name=f"sc{hp}_{i}")
                nc.tensor.matmul(sc[:, 0:SQ],
                                 kT_all[0:HD, gA // 2, i * P:(i + 1) * P],
                                 qT[hp][0:HD, :], start=True, stop=True)
                nc.tensor.matmul(sc[:, SQ:2 * SQ],
                                 kT_all[HD:2 * HD, gB // 2, i * P:(i + 1) * P],
                                 qT[hp][HD:2 * HD, :], start=True, stop=True)
                return sc

            def emit_normalize(hp, ctx0, ctx1):
                # normalize: ctx[c, q] / Z[q]  (Z = ones-column row).
                # Copy psum out FIRST (cheap) so the banks free early for the
                # next head-pair; the broadcast+mult then run off SBUF.
                for half, cps in ((0, ctx0), (1, ctx1)):
                    craw = cr_pool.tile([HD, SQ], F32, tag="cr")
                    nc.vector.tensor_copy(craw[:], cps[0:HD, :])
                    zrow = rc_pool.tile([1, SQ], F32, tag="rc")
                    nc.vector.reciprocal(zrow[:], cps[HD:HD + 1, :])
                    bcv = bc_pool.tile([HD, SQ], F32, tag="bc")
                    nc.gpsimd.partition_broadcast(bcv[:], zrow[:])
                    nc.vector.tensor_tensor(
                        ctx_all[half * HD:(half + 1) * HD, hp, :],
                        craw[:], bcv[:], op=MULT)

            # Flat unit stream over (hp, i).  Per unit: exp(u) on ACT, then
            # scores(u+1) on PE (its psum slot was just freed by exp(u-1)),
            # THEN the AV pair of the PREVIOUS unit, then one interleaved
            # q-proj matmul.  Emitting AV one unit late keeps the freshly
            # unblocked scores matmul at the PE queue head, breaking the
            # sem->AV->scores->sem cycle that would otherwise pace the ACT
            # stream slower than exp itself.
            sc_next = emit_scores(0, 0)
            ctx = {}     # hp -> (ctx0, ctx1)
            qp_cur = None
            pend = None  # (at, i, gA, gB, hp) awaiting AV emission

            def emit_av(at, i, gA, gB, hp):
                ctx0, ctx1 = ctx[hp]
                nc.tensor.matmul(ctx0[0:HD + 1, :],
                                 v_ones[:, i, gA, :], at[:, 0:SQ],
                                 start=(i == 0), stop=(i == NSK - 1))
                nc.tensor.matmul(ctx1[0:HD + 1, :],
                                 v_ones[:, i, gB, :], at[:, SQ:2 * SQ],
                                 start=(i == 0), stop=(i == NSK - 1))

            for u in range(NK * NSK):
                hp, i = divmod(u, NSK)
                gA = HEADS_E[hp] // HPG
                gB = HEADS_O[hp] // HPG
                if i == 0:
                    ctx[hp] = (psA.tile([P, SQ], F32, tag="ps", name=f"ctx0_{hp}"),
                               psA.tile([P, SQ], F32, tag="ps", name=f"ctx1_{hp}"))
                    if hp + 2 < NK:
                        qp_cur = emit_qproj_mm(hp + 2)
                    else:
                        qp_cur = None
                sc = sc_next
                at = attn_pool.tile([P, 2 * SQ], BF16, tag="at")
                nc.scalar.activation(at[:], sc[:], EXP,
                                     bias=mb_sb[:, i:i + 1], scale=0.125)
                if u + 1 < NK * NSK:
                    nhp, ni = divmod(u + 1, NSK)
                    sc_next = emit_scores(nhp, ni)
                if pend is not None:
                    emit_av(*pend)
                    if pend[1] == NSK - 1:   # that AV closed head-pair hp-1
                        php = pend[4]
                        emit_normalize(php, *ctx.pop(php))
                        qT.pop(php, None)
                pend = (at, i, gA, gB, hp)
                if qp_cur is not None:
                    emit_qp_step(qp_cur[0], qp_cur[1], i)
                    if i == NSK - 1:
                        emit_qproj_fin(hp + 2, qp_cur[0])
            emit_av(*pend)
            emit_normalize(NK - 1, *ctx.pop(NK - 1))

            # ---- out projection (transposed): out^T = o_w^T @ ctx^T ----
            for mh in range(8):  # half-groups of 256 output cols
                owt = ow_pool.tile([P, NK, 2 * P], BF16, tag="ow", name=f"owt{mh}")
                nc.sync.dma_start(owt[:], ow_d.ap()[:, mh * 2 * P:(mh + 1) * 2 * P]
                                  .rearrange("(k p) c -> p k c", p=P))
                ps_o = [psA.tile([P, SQ], F32, tag="ps", name=f"ps_o{mh}_{mj}")
                        for mj in range(2)]
                for k in range(NK):
                    for mj in range(2):
                        nc.tensor.matmul(ps_o[mj][:], owt[:, k, mj * P:(mj + 1) * P],
                                         ctx_all[:, k, :],
                                         start=(k == 0), stop=(k == NK - 1))
                for mj in range(2):
                    m = mh * 2 + mj
                    ot = ev_pool.tile([P, SQ], F32, tag="ot")
                    nc.vector.tensor_scalar_add(ot[:], ps_o[mj][:], ob_sb[:, m:m + 1])
                    nc.sync.dma_start(outT_d.ap()[m * P:(m + 1) * P, :], ot[:])

    nc.compile()
    return nc


def _get_nc():
    if "nc" not in _CACHE:
        _CACHE["nc"] = _build()
    return _CACHE["nc"]


def prep_in_maps(x, mask, q_w, q_b, k_w, k_b, v_w, v_b, o_w, o_b):
    """Host-side sharding: returns the 8 per-core input dicts."""
    bf = ml_dtypes.bfloat16
    x = np.asarray(x, np.float32)
    mask = np.asarray(mask)
    q_w = np.asarray(q_w, np.float32)
    q_b = np.asarray(q_b, np.float32)
    o_w = np.asarray(o_w, np.float32)
    v_b = np.asarray(v_b, np.float32)

    # head permutation: tile hp = (HEADS_E[hp], HEADS_O[hp])
    col_perm = np.zeros(D, np.int64)
    for hp in range(NK):
        col_perm[hp * P:hp * P + HD] = np.arange(HEADS_E[hp] * HD,
                                                 (HEADS_E[hp] + 1) * HD)
        col_perm[hp * P + HD:(hp + 1) * P] = np.arange(HEADS_O[hp] * HD,
                                                       (HEADS_O[hp] + 1) * HD)
    qw = np.ascontiguousarray(q_w[:, col_perm]).astype(bf)
    qb = np.ascontiguousarray(q_b[col_perm]).reshape(D, 1)
    ow = np.ascontiguousarray(o_w[col_perm, :]).astype(bf)

    kw = np.asarray(k_w, np.float32).astype(bf)
    vw = np.asarray(v_w, np.float32).astype(bf)
    kb = np.asarray(k_b, np.float32).reshape(KV, 1)
    ob = np.asarray(o_b, np.float32).reshape(D, 1)

    vbo = np.zeros((P, G * (HD + 1)), np.float32)
    for g in range(G):
        vbo[:, g * (HD + 1):g * (HD + 1) + HD] = v_b[g * HD:(g + 1) * HD][None, :]
        vbo[:, g * (HD + 1) + HD] = 1.0
    vbo = vbo.astype(bf)

    xT = [np.ascontiguousarray(x[b].T).astype(bf) for b in range(B)]
    mbias = [np.where(np.asarray(mask[b]) == 0, np.float32(-30000.0),
                      np.float32(0.0)) for b in range(B)]

    in_maps = []
    for c in range(N_CORES):
        b, q4 = c // 4, c % 4
        # rotate so own query quarter is key-chunk 0
        xTr = np.ascontiguousarray(np.roll(xT[b], -q4 * SQ, axis=1))
        mbr = np.ascontiguousarray(np.roll(mbias[b], -q4 * SQ)).reshape(S, 1)
        # boot tile: [p, (t c)] with t=0 -> x^T k-chunk 0 (own quarter),
        # t=1 -> k_w k-chunk 0 -- one DMA feeds the first matmul
        boot = np.concatenate([xTr[0:P, 0:SQ], kw[0:P, :]],
                              axis=1)  # [P, 2*SQ]
        boot = np.ascontiguousarray(boot)
        in_maps.append({
            "xT": xTr,
            "qw": qw, "kw": kw, "vw": vw, "ow": ow,
            "qb": qb, "kb": kb, "ob": ob,
            "mb": mbr, "vbo": vbo, "boot": boot,
        })
    return in_maps


def kernel(x, mask, q_w, q_b, k_w, k_b, v_w, v_b, o_w, o_b):
    in_maps = prep_in_maps(x, mask, q_w, q_b, k_w, k_b, v_w, v_b, o_w, o_b)
    nc = _get_nc()
    res = bass_utils.run_bass_kernel_spmd(nc, in_maps, core_ids=list(range(N_CORES)))
    out = np.empty((B, S, D), np.float32)
    for c in range(N_CORES):
        b, q4 = c // 4, c % 4
        out[b, q4 * SQ:(q4 + 1) * SQ, :] = res.results[c]["outT"].T
    return out
